# revision 27
# baseline (speedup 1.0000x reference)
"""Taylor feature map kernel for Trainium2 (Bass/Tile), 8-core SPMD.

Input  x:   (2, 16, 2048, 64) f32  -> 65536 rows of dim 64
Output out: (2, 16, 2048, 2145) f32 per row:
    [1, x/D^0.25, x_i^2/(sqrt(D)*sqrt(2)), x_i*x_j/sqrt(D) for i<j (row-major)]

Sharding: rows are purely elementwise -> split 65536 rows into 8 contiguous
chunks of 8192, one per NeuronCore. No communication.

Per-core layout: row r = p*64 + g (partition p, row-group g); supertiles
cover group ranges. Each supertile's full 2145-col output lives in ONE SBUF
tile [128, G, 2145] so the store DMA sees per-partition-contiguous runs of
(G/2)*2145*4 bytes (34KB descriptors instead of 4KB per-row ones); that
lifts the sync-ring store stream from ~330 to ~400 GB/s. Supertile sizes
ramp up [2,2,3,4,6,7,8...] so the first store issues early.

Engine assignment per supertile (three hard-won port facts drive it):
  ACT    ones col 0, linear cols [1,65), diag cols [65,129), and the DVE
         band's y operand into PSUM (own SBUF ports, off the band path)
  DVE    cross jobs [E,63): in0 broadcast from SBUF y (dedicated port),
         in1 streamed from PSUM y  -> takes only ONE SBUF source
  GPSIMD cross jobs [0,E) reading SBUF y
1) DVE+GpSimd share one SBUF port pair, lock-held per instruction — a
   2-SBUF-source DVE op blocks GpSimd entirely, so DVE reads its streaming
   operand from PSUM (exactly one PSUM operand is legal for TensorTensor).
2) The Tile dep tracker keeps exact ranges only for <=3D APs; pair-op out
   APs are pre-flattened to 3D or the bands falsely serialize.
3) Stores stay on the sync HWDGE ring: DMA issues on the scalar ring wait
   in the ACT FIFO behind compute sems and wreck the stream. Loads (no
   deps) ride the scalar ring.
Cross jobs are mostly paired ops (2 jobs/op); each pair's one garbage-lane
element lands on the next job's first column and is overwritten by the next
op on the same engine, so the last jobs of each band are singles to keep
garbage inside the band's column range; the garbage-lane READ hits the y
scratch's padded 65th column.
"""

import math
from contextlib import ExitStack

import numpy as np

try:
    import concourse.bass as bass
except ImportError:  # container path for the concourse framework
    import sys

    sys.path.insert(0, "/opt/trn_rl_repo")
    import concourse.bass as bass

import concourse.mybir as mybir
from concourse import tile
from concourse.bass_utils import run_bass_kernel_spmd
from concourse.vector_clock import ScopedClock

MAX_WAITS = 1


class SplitWaitTileContext(tile.TileContext):
    """The stock walrus in this environment rejects instructions carrying
    more than one sync wait ("Too many sync wait commands", observed for
    both TPB_CTRL Drain and DMA_DIRECT2D). Hoist excess waits onto NoOp
    carrier instructions committed just before, on the same engine queue."""

    def _split_waits(self, inst):
        si = getattr(inst, "sync_info", None)
        eng = getattr(inst, "engine", None)
        if (
            si is None
            or not si.on_wait
            or len(si.on_wait) <= MAX_WAITS
            or eng is None
            or eng == mybir.EngineType.Unassigned
        ):
            return None
        waits = list(si.on_wait)
        extra, keep = waits[:-MAX_WAITS], waits[-MAX_WAITS:]
        inst.sync_info = mybir.SyncInfo(on_wait=keep,
                                        on_update=list(si.on_update))
        nops = []
        for i in range(0, len(extra), MAX_WAITS):
            nops.append(mybir.InstNoOp(
                name=self.nc.get_next_instruction_name(),
                sync_info=mybir.SyncInfo(on_wait=extra[i:i + MAX_WAITS],
                                         on_update=[]),
                bass_nofuse=True,
                engine=eng,
            ))
        return nops

    def _commit_instruction(self, inst, lazy_reg_writes=True):
        if isinstance(inst, mybir.Instruction):
            nops = self._split_waits(inst)
            if nops:
                for nop in nops:
                    super()._commit_instruction(nop)
        return super()._commit_instruction(inst, lazy_reg_writes)

    def _drain_and_barrier(self, tick_clock, wait_clock):
        nc = self.nc
        drain_inst = nc.sync.drain()
        wait_clock.add_sem_waits(
            drain_inst.ins, ScopedClock({None: tick_clock.global_clock})
        )
        nops = self._split_waits(drain_inst.ins)
        if nops:
            # _commit path is closed here; append carriers directly, then
            # re-emit a drain that executes after them on the same queue.
            for nop in nops:
                self._add_instruction(nop)
            nc.sync.drain()

        nc.all_engine_barrier()
        assert self.sems is not None
        popped = nc._tile_sem_poison_stack.pop()
        assert popped is self._sem_poison
        nc.clear_and_free_semaphores(list(self.sems.allocated().values()))
        nc.all_engine_barrier()

D = 64
N_CROSS = (D * (D - 1)) // 2  # 2016
OUT_D = 1 + D + D + N_CROSS   # 2145
P = 128
N_CORES = 8
ROWS_TOTAL = 2 * 16 * 2048    # 65536
ROWS_PER_CORE = ROWS_TOTAL // N_CORES  # 8192

RD = math.sqrt(D)                      # 8.0
RRD_INV = 1.0 / math.sqrt(RD)          # 1/D^0.25; note (1/rrd)^2 == 1/rd
DIAG_C = 1.0 / math.sqrt(RD * math.sqrt(2.0))  # (c*x)^2 = x^2/(rd*sqrt2)

BASE = 1 + 2 * D  # 129, start of cross block

_OFF = [0] * 64
for _i in range(63):
    _OFF[_i + 1] = _OFF[_i] + (63 - _i)

# (groups, gpsimd/DVE job split) per supertile; groups sum to 64
# (8192 rows / 128 partitions). Ramped so the first store issues early.
# DVE and GpSimd share one SBUF port pair (lock per instruction), so the
# DVE band reads its operands from PSUM instead — then the bands really
# run in parallel. E balances gpsimd (~2.1 ns/elem) vs DVE (~1.3).
RAMP = [(2, 24), (2, 24), (3, 21), (4, 18), (6, 17), (7, 15),
        (8, 15), (8, 15), (8, 15), (8, 15), (8, 15)]
G_MAX = 8
G_TOTAL = 64  # total row-groups per core; x is held as one [P, 64, D] tile
# x arrives in a few chunked DMAs so early tiles' data lands early
X_CHUNKS = [(0, 2), (2, 6), (6, 12), (12, 28), (28, 64)]


YS_W = D + 1  # y scratch width: 64 y columns + 1 defined pad column


def _pair_aps(t_sb, y0_sb, y1_sb, groups, i):
    """Access patterns computing cross jobs i and i+1 in one op.

    out[p,g,q,j] = y_{i+q} * y_{i+q+1+j},  q in {0,1}, j in [0, 63-i).
    Job i+1's run is padded by one garbage element which lands on
    off(i+2)[0] and is overwritten by the next op on the same engine.
    Writes go to the merged tile t_sb; in0 reads from y0_sb, in1 from
    y1_sb (width YS_W; the last, padded column absorbs the garbage
    lane's read so bands never touch t_sb's diag columns). For the DVE
    band y1_sb lives in PSUM so the op takes only one SBUF source and
    never locks the DVE/gpsimd shared SBUF port pair.
    """
    n = 63 - i
    t0 = t_sb[:, :, 0:1]
    pstep = t0.ap[0][0]
    y0 = y0_sb[:, :, 0:1]
    y1 = y1_sb[:, :, 0:1]
    # 3D (pre-flattened [n,2],[1,n] -> [1,2n]) so the dep tracker keeps
    # exact per-group column ranges; 4D APs fall back to bounding-range
    # tracking, which falsely conflicts every pair op of both bands and
    # serializes them.
    out = bass.AP(t0.tensor, BASE + _OFF[i],
                  [[pstep, P], [OUT_D, groups], [1, 2 * n]])
    in0 = bass.AP(y0.tensor, i,
                  [[y0.ap[0][0], P], [YS_W, groups], [1, 2], [0, n]])
    in1 = bass.AP(y1.tensor, 1 + i,
                  [[y1.ap[0][0], P], [YS_W, groups], [1, 2], [1, n]])
    return out, in0, in1


def _emit_band(nc_eng, t_sb, y0_sb, y1_sb, groups, i_beg, i_end):
    """Cross jobs [i_beg, i_end) on one engine: pairs, then singles for the
    last 2-3 jobs so pair garbage never crosses i_end's column boundary."""
    i = i_beg
    while i < i_end:
        if i + 3 < i_end:
            o_ap, a_ap, b_ap = _pair_aps(t_sb, y0_sb, y1_sb, groups, i)
            nc_eng.tensor_mul(o_ap, a_ap, b_ap)
            i += 2
        else:
            n = 63 - i
            dst = t_sb[:, :, BASE + _OFF[i]: BASE + _OFF[i] + n]
            a = y0_sb[:, :, i: 1 + i].broadcast_to((P, groups, n))
            nc_eng.tensor_mul(dst, a, y1_sb[:, :, 1 + i: 1 + i + n])
            i += 1


def build_nc(rows_per_core: int = ROWS_PER_CORE) -> bass.Bass:
    assert sum(g for g, _ in RAMP) * P == rows_per_core

    nc = bass.Bass()
    x = nc.declare_dram_parameter("x", [rows_per_core, D], mybir.dt.float32,
                                  isOutput=False)
    out = nc.declare_dram_parameter("out", [rows_per_core, OUT_D],
                                    mybir.dt.float32, isOutput=True)

    f32 = mybir.dt.float32
    copy_fn = mybir.ActivationFunctionType.Copy
    square_fn = mybir.ActivationFunctionType.Square

    with SplitWaitTileContext(nc) as tc, ExitStack() as ctx:
        xp = ctx.enter_context(tc.tile_pool(name="xp", bufs=1))
        op = ctx.enter_context(tc.tile_pool(name="op", bufs=2))
        yp = ctx.enter_context(tc.tile_pool(name="yp", bufs=2))
        pp = ctx.enter_context(tc.psum_pool(name="pp", bufs=2))

        # y scratch rings (2 persistent tiles each): DVE's band reads from
        # PSUM (its own ports — no shared-pair lock), gpsimd's from SBUF.
        # The padded last column (read by the pair ops' garbage lane) is
        # written once and never rewritten.
        y_dve, y_gp = [], []
        for _ in range(2):
            y_sb = yp.tile([P, G_MAX, YS_W], f32, tag="y")
            nc.gpsimd.memset(y_sb[:, :, D:YS_W], 0.0)
            y_gp.append(y_sb)
            y_ps = pp.tile([P, G_MAX, YS_W], f32, tag="yp")
            nc.scalar.activation(y_ps[:, :, D:YS_W], y_sb[:, :, D:YS_W],
                                 copy_fn, bias=0.0, scale=0.0)
            y_dve.append(y_ps)

        # one global x tile: row r = p*G_TOTAL + g. Loaded in a few chunks
        # on the scalar HWDGE ring (no compute deps — they issue before any
        # ACT compute op) so the sync ring carries only stores.
        x_all = xp.tile([P, G_TOTAL, D], f32)
        xr = x[:, :].rearrange("(p g) d -> p g d", g=G_TOTAL)
        for c0, c1 in X_CHUNKS:
            nc.scalar.dma_start(x_all[:, c0:c1, :], xr[:, c0:c1, :])

        out_r = out[:, :].rearrange("(p g) d -> p g d", g=G_TOTAL)
        col = [0]
        for gk, _ in RAMP:
            col.append(col[-1] + gk)

        def fill_y(k):
            """y = x / D^0.25 for tile k. DVE's copy lives in PSUM, filled
            by ACT; gpsimd's SBUF copy is filled by DVE's one-source
            tensor_scalar (dedicated ports only). Emitted one tile ahead
            so the ACT-queue store issue never gates the next band."""
            gk = RAMP[k][0]
            xv = x_all[:, col[k]:col[k] + gk, :]
            nc.scalar.activation(y_dve[k % 2][:, 0:gk, 0:D], xv,
                                 copy_fn, scale=RRD_INV)
            nc.vector.tensor_scalar_mul(y_gp[k % 2][:, 0:gk, 0:D], xv,
                                        RRD_INV)

        fill_y(0)
        for k, (gk, ek) in enumerate(RAMP):
            c0, c1 = col[k], col[k + 1]
            xv = x_all[:, c0:c1, :]
            a_full = op.tile([P, G_MAX, OUT_D], f32, tag="o")
            a_sb = a_full[:, 0:gk, :]
            yd = y_dve[k % 2][:, 0:gk, :]
            yg = y_gp[k % 2][:, 0:gk, :]
            # linear block of the output: y values, written by ACT
            nc.scalar.activation(a_sb[:, :, 1:1 + D], xv,
                                 copy_fn, scale=RRD_INV)
            # diag block: Square(c*x) = x^2/(rd*sqrt2)  (cols 65..129, ACT)
            nc.scalar.activation(a_sb[:, :, 1 + D:1 + 2 * D], xv,
                                 square_fn, scale=DIAG_C)
            # ones column (ACT): Copy(0*x + 1)
            nc.scalar.activation(a_sb[:, :, 0:1], xv[:, :, 0:1],
                                 copy_fn, bias=1.0, scale=0.0)
            # cross bands: gpsimd reads SBUF y only; DVE reads its
            # broadcast operand from SBUF (dedicated port) and its
            # streaming operand from PSUM — the bands share no port.
            _emit_band(nc.gpsimd, a_sb, yg, yg, gk, 0, ek)
            _emit_band(nc.vector, a_sb, yg, yd, gk, ek, 63)
            if k + 1 < len(RAMP):
                fill_y(k + 1)

            # store in two group-halves (keeps descriptors under 64KB;
            # single whole-tile DMAs measured slower)
            if gk == 1:
                nc.sync.dma_start(out_r[:, c0:c1, :], a_sb[:, 0:gk, :])
            else:
                gh = gk // 2
                nc.sync.dma_start(out_r[:, c0:c0 + gh, :], a_sb[:, 0:gh, :])
                nc.sync.dma_start(out_r[:, c0 + gh:c1, :],
                                  a_sb[:, gh:gk, :])
    return nc


_NC_CACHE: dict = {}


def _install_ntff_hook_shim():
    """The image's antenv lacks axon_hooks; provide it so trace=True can
    drive NRT profiling via ctypes into libaxon_pjrt.so."""
    import sys as _sys
    import types
    import ctypes
    import contextlib

    if "antenv.axon_hooks" in _sys.modules:
        return
    so_path = "/opt/axon/libaxon_pjrt.so"
    lib = ctypes.CDLL(so_path)
    if not hasattr(lib, "axon_start_nrt_profile"):
        return
    lib.axon_start_nrt_profile.argtypes = [
        ctypes.POINTER(ctypes.c_int64), ctypes.c_size_t]
    lib.axon_start_nrt_profile.restype = ctypes.c_int64
    lib.axon_stop_nrt_profile.argtypes = [ctypes.c_char_p]
    lib.axon_stop_nrt_profile.restype = ctypes.c_int64

    @contextlib.contextmanager
    def _hook(output_dir, device_ids):
        import jax
        jax.devices()
        if device_ids:
            ids = (ctypes.c_int64 * len(device_ids))(*device_ids)
            rc = lib.axon_start_nrt_profile(ids, len(device_ids))
        else:
            rc = lib.axon_start_nrt_profile(None, 0)
        if rc != 0:
            raise RuntimeError(f"axon_start_nrt_profile rc={rc}")
        try:
            yield
        finally:
            n = lib.axon_stop_nrt_profile(str(output_dir).encode())
            print(f"ntff profile: {n} file(s) written to {output_dir}")

    mod = types.ModuleType("antenv.axon_hooks")
    mod.set_axon_ntff_profile_hook = lambda h: None
    mod.get_axon_ntff_profile_hook = lambda: _hook
    _sys.modules["antenv.axon_hooks"] = mod
    import antenv
    antenv.axon_hooks = mod


def _get_nc():
    if "nc" not in _NC_CACHE:
        _NC_CACHE["nc"] = build_nc()
    return _NC_CACHE["nc"]


def _install_loud_cc_hook():
    """Surface the real python traceback when the PJRT compile callback
    fails (the C++ caller swallows it)."""
    from concourse import bass2jax
    bass2jax.install_neuronx_cc_hook()
    try:
        import libneuronxla
    except ImportError:
        return
    if getattr(libneuronxla, "_loud_wrapped", False):
        return
    orig = libneuronxla.neuronx_cc

    def loud_hook(*a, **kw):
        try:
            return orig(*a, **kw)
        except BaseException:
            import traceback
            import sys as _s
            traceback.print_exc()
            _s.stderr.flush()
            raise

    libneuronxla.neuronx_cc = loud_hook
    libneuronxla._loud_wrapped = True
    bass2jax.install_neuronx_cc_hook = lambda: None


def _run(x_np: np.ndarray, trace: bool = False):
    _install_loud_cc_hook()
    if trace:
        _install_ntff_hook_shim()
    nc = _get_nc()
    in_maps = [{"x": x_np[c * ROWS_PER_CORE:(c + 1) * ROWS_PER_CORE]}
               for c in range(N_CORES)]
    res = run_bass_kernel_spmd(nc, in_maps, list(range(N_CORES)), trace=trace)
    out = np.concatenate([res.results[c]["out"] for c in range(N_CORES)],
                         axis=0)
    return out, res


def kernel(x) -> np.ndarray:
    x_np = np.ascontiguousarray(np.asarray(x), dtype=np.float32)
    shape = x_np.shape
    x_np = x_np.reshape(ROWS_TOTAL, D)
    out, _ = _run(x_np, trace=False)
    return out.reshape(*shape[:-1], OUT_D)
